# revision 3
# baseline (speedup 1.0000x reference)
"""Trainium2 Bass kernel for nn_Mnist_lmdSplineKAN.

Sharding: data-parallel over batch, 8 cores x 128 rows. All params replicated.

Per-core math (I=784 inputs, H=10 heads, O=64, 8 B-spline basis fns, order 3,
5 uniform intervals on [0,1)):
  features[b,i,0:8] = 6*bspline_basis(x)[b,i,:]   (one-hot interval masks x
                      local cubic polys, computed on DVE/ACT in fp16)
  features[b,i,8]   = silu(x[b,i])
  y[b,(h,o)] = sum_{i,j} features[b,i,j] * Wbig[(i,j),(h,o)]   (fp16 matmul,
                      Wbig folds coef*scale_sp*lmd/6 and scale_base*lmd)
  h1 = tanh(y);  h2 = tanh(h1 @ blockdiag(W1) + b1);  logits = <h2,W2>_head + b2
"""
import sys, types
import numpy as np

B, I, O, H, NB = 1024, 784, 64, 10, 8  # NB = basis fns
NC = 8            # cores
BC = B // NC      # 128 batch rows per core
P = 112           # partition block of I
CH = I // P       # 7 chunks
HO = H * O        # 640
D2 = H * 32       # 320
NH = 2            # N-halves of HO for PSUM banks (320 each)


def _install_ntff_hook():
    """antenv.axon_hooks is absent on this image; recreate it so
    run_bass_kernel_spmd(trace=True) can profile via the axon .so."""
    if "antenv.axon_hooks" in sys.modules:
        return
    try:
        import antenv
        mod = types.ModuleType("antenv.axon_hooks")
        _h = [None]
        mod.set_axon_ntff_profile_hook = lambda h: _h.__setitem__(0, h)
        mod.get_axon_ntff_profile_hook = lambda: _h[0]
        sys.modules["antenv.axon_hooks"] = mod
        antenv.axon_hooks = mod
        from trn_agent_boot.trn_boot import _ntff_profile_via_ctypes
        h = _ntff_profile_via_ctypes("/opt/axon/libaxon_pjrt.so")
        if h is not None:
            mod.set_axon_ntff_profile_hook(h)
    except Exception:
        pass


_CACHE = {}


def _build():
    if "nc" in _CACHE:
        return _CACHE["nc"]
    import concourse.bacc as bacc
    import concourse.bass as bass
    import concourse.tile as tile
    from concourse import mybir
    from contextlib import ExitStack

    f32, f16 = mybir.dt.float32, mybir.dt.float16
    ALU = mybir.AluOpType
    AF = mybir.ActivationFunctionType

    nc = bacc.Bacc("TRN2", target_bir_lowering=False, debug=False)
    x_d = nc.dram_tensor("x", (P, CH, BC), f32, kind="ExternalInput").ap()
    w_d = nc.dram_tensor("w", (P, CH, NB + 1, HO), f16, kind="ExternalInput").ap()
    w1_d = nc.dram_tensor("w1", (128, 5, D2), f16, kind="ExternalInput").ap()
    b1_d = nc.dram_tensor("b1", (D2,), f32, kind="ExternalInput").ap()
    w2_d = nc.dram_tensor("w2", (D2,), f32, kind="ExternalInput").ap()
    b2_d = nc.dram_tensor("b2", (H,), f32, kind="ExternalInput").ap()
    out_d = nc.dram_tensor("out", (BC, H), f32, kind="ExternalOutput").ap()

    GRID = (np.arange(-3, 9) * 0.2).astype(np.float32)  # knots, match reference
    THR = [float(GRID[3 + t]) for t in range(1, 5)]     # 0.2 0.4 0.6 0.8

    def bcast(dram_ap, n):
        return bass.AP(tensor=dram_ap.tensor, offset=dram_ap.offset,
                       ap=[[0, 128]] + [[1, n]])

    with tile.TileContext(nc) as tc, ExitStack() as ctx:
        sb = ctx.enter_context(tc.tile_pool(name="sb", bufs=1))
        ps = ctx.enter_context(tc.tile_pool(name="ps", bufs=1, space="PSUM"))

        # ---- DMAs in ----
        xt = sb.tile([P, CH, BC], f32, tag="xt")
        nc.sync.dma_start(xt[:], x_d)
        wc = []
        for c in range(CH):
            t = sb.tile([P, NB + 1, HO], f16, tag=f"wc{c}")
            nc.sync.dma_start(t[:], w_d[:, c, :, :])
            wc.append(t)
        w1t = sb.tile([128, 5, D2], f16, tag="w1t")
        nc.sync.dma_start(w1t[:], w1_d)
        b1b = sb.tile([128, D2], f32, tag="b1b")
        nc.sync.dma_start(b1b[:], bcast(b1_d, D2))
        w2b = sb.tile([128, D2], f32, tag="w2b")
        nc.sync.dma_start(w2b[:], bcast(w2_d, D2))
        b2b = sb.tile([128, H], f32, tag="b2b")
        nc.sync.dma_start(b2b[:], bcast(b2_d, H))

        x = xt[:].rearrange("p c b -> p (c b)")  # (112, 896) view

        def T(tag, dt=f16):
            return sb.tile([P, CH * BC], dt, tag=tag, name=tag)

        # ---- masks (DVE) ----
        c_ = [None] * 5
        for t in range(1, 5):
            c_[t] = T(f"c{t}")
            nc.vector.tensor_scalar(c_[t][:], x, THR[t - 1], None, op0=ALU.is_ge)
        m = [None] * 5
        m[0] = T("m0")
        nc.vector.tensor_scalar(m[0][:], c_[1][:], -1.0, 1.0, op0=ALU.mult, op1=ALU.add)
        for t in (1, 2, 3):
            m[t] = T(f"m{t}")
            nc.vector.tensor_tensor(m[t][:], c_[t][:], c_[t + 1][:], op=ALU.subtract)
        m[4] = c_[4]

        # ---- t-index and u = 5x - t (fp32) ----
        s12 = T("s12"); nc.vector.tensor_tensor(s12[:], c_[1][:], c_[2][:], op=ALU.add)
        s34 = T("s34"); nc.vector.tensor_tensor(s34[:], c_[3][:], c_[4][:], op=ALU.add)
        tm = T("tm"); nc.vector.tensor_tensor(tm[:], s12[:], s34[:], op=ALU.add)
        v = T("v", f32)
        nc.scalar.activation(v[:], x, AF.Copy, bias=0.0, scale=5.0)
        u = T("u", f32)
        nc.vector.tensor_tensor(u[:], v[:], tm[:], op=ALU.subtract)

        # ---- local cubics (x6-scaled): p0=(1-u)^3 p1=3u^3-6u^2+4
        #      p2=p1(1-u) p3=u^3 ----
        u2 = T("u2", f32); nc.scalar.activation(u2[:], u[:], AF.Square)
        w_ = T("w_", f32)
        nc.vector.tensor_scalar(w_[:], u[:], -1.0, 1.0, op0=ALU.mult, op1=ALU.add)
        w2_ = T("w2_", f32); nc.scalar.activation(w2_[:], w_[:], AF.Square)
        p3h = T("p3h"); nc.vector.tensor_tensor(p3h[:], u2[:], u[:], op=ALU.mult)
        p0h = T("p0h"); nc.vector.tensor_tensor(p0h[:], w2_[:], w_[:], op=ALU.mult)
        a_ = T("a_", f32)
        nc.vector.tensor_scalar(a_[:], u[:], 3.0, -6.0, op0=ALU.mult, op1=ALU.add)
        p1f = T("p1f", f32)
        nc.vector.tensor_tensor(p1f[:], a_[:], u2[:], op=ALU.mult)
        p1h = T("p1h")
        nc.vector.tensor_scalar(p1h[:], p1f[:], 1.0, 4.0, op0=ALU.mult, op1=ALU.add)
        b_ = T("b_", f32)
        nc.vector.tensor_scalar(b_[:], w_[:], 3.0, -6.0, op0=ALU.mult, op1=ALU.add)
        p2f = T("p2f", f32)
        nc.vector.tensor_tensor(p2f[:], b_[:], w2_[:], op=ALU.mult)
        p2h = T("p2h")
        nc.vector.tensor_scalar(p2h[:], p2f[:], 1.0, 4.0, op0=ALU.mult, op1=ALU.add)
        ph = [p0h, p1h, p2h, p3h]

        # ---- feature maps F_j = sum_t m_t * p_{j-t}; F_8 = silu ----
        f_ = []
        for j in range(NB):
            f_.append(sb.tile([P, CH, BC], f16, tag=f"f{j}", name=f"f{j}"))
        fs = sb.tile([P, CH, BC], f16, tag="f8")
        nc.scalar.activation(fs[:].rearrange("p c b -> p (c b)"), x, AF.Silu)
        f_.append(fs)

        psum = [ps.tile([128, D2], f32, tag=f"y{nh}", name=f"y{nh}") for nh in range(NH)]
        nmm = [0, 0]

        def emit_mms(j):
            for c in range(CH):
                for nh in range(NH):
                    nc.tensor.matmul(
                        psum[nh][:],
                        f_[j][:, c, :],
                        wc[c][:, j, nh * D2:(nh + 1) * D2],
                        start=(nmm[nh] == 0),
                        stop=(nmm[nh] == CH * (NB + 1) - 1),
                    )
                    nmm[nh] += 1

        # j emission order: cheapest features first so PE starts early
        tmp = T("tmp")
        tmp2 = T("tmp2")
        for j in (0, 7, 1, 6, 2, 5, 3, 4):
            terms = [(t, j - t) for t in range(5) if 0 <= j - t <= 3]
            out = f_[j][:].rearrange("p c b -> p (c b)")
            if len(terms) == 1:
                t, r = terms[0]
                nc.vector.tensor_tensor(out, m[t][:], ph[r][:], op=ALU.mult)
            else:
                acc = tmp[:]
                t, r = terms[0]
                nc.vector.tensor_tensor(acc, m[t][:], ph[r][:], op=ALU.mult)
                for k, (t, r) in enumerate(terms[1:]):
                    pr = tmp2[:]
                    nc.vector.tensor_tensor(pr, m[t][:], ph[r][:], op=ALU.mult)
                    dst = out if k == len(terms) - 2 else acc
                    nc.vector.tensor_tensor(dst, acc, pr, op=ALU.add)
            emit_mms(j)
        emit_mms(NB)  # silu feature

        # ---- tail: h1 = tanh(y) ----
        h1 = sb.tile([128, HO], f16, tag="h1")
        for nh in range(NH):
            nc.scalar.activation(h1[:, nh * D2:(nh + 1) * D2], psum[nh][:], AF.Tanh)
        # transpose h1 into 5 (128,128) tiles via DMA transpose
        h1t = []
        for k in range(5):
            t = sb.tile([128, 128], f16, tag=f"h1t{k}")
            nc.sync.dma_start_transpose(t[:], h1[:, k * 128:(k + 1) * 128])
            h1t.append(t)
        ps2 = ps.tile([128, D2], f32, tag="ps2")
        for k in range(5):
            nc.tensor.matmul(ps2[:], h1t[k][:], w1t[:, k, :],
                             start=(k == 0), stop=(k == 4))
        h2a = sb.tile([128, D2], f32, tag="h2a")
        nc.vector.tensor_tensor(h2a[:], ps2[:], b1b[:], op=ALU.add)
        h2 = sb.tile([128, D2], f32, tag="h2")
        nc.scalar.activation(h2[:], h2a[:], AF.Tanh)
        prod = sb.tile([128, D2], f32, tag="prod")
        nc.vector.tensor_tensor(prod[:], h2[:], w2b[:], op=ALU.mult)
        red = sb.tile([128, H], f32, tag="red")
        nc.vector.tensor_reduce(red[:], prod[:].rearrange("p (h d) -> p h d", d=32),
                                axis=mybir.AxisListType.X, op=ALU.add)
        lg = sb.tile([128, H], f32, tag="lg")
        nc.vector.tensor_tensor(lg[:], red[:], b2b[:], op=ALU.add)
        nc.sync.dma_start(out_d, lg[:])

    nc.compile()
    _CACHE["nc"] = nc
    return nc


def _prep_inputs(x, coef, scale_base, scale_sp, lmd, W1, b1, W2, b2):
    xf = np.asarray(x, np.float64).reshape(B, I)
    coef = np.asarray(coef, np.float64)
    eff = coef * np.asarray(scale_sp, np.float64)[..., None] \
        * np.asarray(lmd, np.float64)[:, :, None, None] / 6.0       # (H,I,O,8)
    sbl = np.asarray(scale_base, np.float64) \
        * np.asarray(lmd, np.float64)[:, :, None]                    # (H,I,O)
    wbig = np.concatenate([eff, sbl[..., None]], -1)                 # (H,I,O,9)
    wdev = wbig.reshape(H, CH, P, O, NB + 1).transpose(2, 1, 4, 0, 3) \
        .reshape(P, CH, NB + 1, HO).astype(np.float16)
    W1 = np.asarray(W1, np.float64)
    w1bd = np.zeros((HO, D2))
    for h in range(H):
        w1bd[h * O:(h + 1) * O, h * 32:(h + 1) * 32] = W1[h]
    w1dev = w1bd.reshape(5, 128, D2).transpose(1, 0, 2).astype(np.float16)
    b1c = np.asarray(b1, np.float32).reshape(D2).copy()
    w2c = np.asarray(W2, np.float32).reshape(D2).copy()
    b2c = np.asarray(b2, np.float32).reshape(H).copy()

    in_maps = []
    for core in range(NC):
        xs = xf[core * BC:(core + 1) * BC].T                          # (784,128)
        xdev = np.ascontiguousarray(
            xs.reshape(CH, P, BC).transpose(1, 0, 2)).astype(np.float32)
        in_maps.append({"x": xdev, "w": wdev, "w1": w1dev,
                        "b1": b1c, "w2": w2c, "b2": b2c})
    return in_maps


def run(inputs, trace=False, tmpdir=None):
    _install_ntff_hook()
    from concourse.bass_utils import run_bass_kernel_spmd
    nc = _build()
    in_maps = _prep_inputs(**inputs)
    res = run_bass_kernel_spmd(nc, in_maps, core_ids=list(range(NC)),
                               trace=trace, tmpdir=tmpdir)
    out = np.concatenate([r["out"] for r in res.results], 0)
    return out.astype(np.float32), res


def kernel(**inputs):
    out, _ = run(inputs)
    return out


# revision 4
# speedup vs baseline: 1.0604x; 1.0604x over previous
"""Trainium2 Bass kernel for nn_Mnist_lmdSplineKAN.

Sharding: data-parallel over batch, 8 cores x 128 rows. All params replicated.

Per-core math (I=784 inputs, H=10 heads, O=64, 8 B-spline basis fns, order 3,
5 uniform intervals on [0,1)):
  t = floor(5x) (int-round trick), u = 5x - t, one-hot masks m_t = (t == const)
  features[b,i,j] = sum_t m_t * p_{j-t}(u)  with p = 6x local cubic polys
  features[b,i,8] = silu(x[b,i])
  y[b,(h,o)] = sum_{i,j} features[b,i,j] * Wbig[(i,j),(h,o)]  (fp16 matmul;
               Wbig folds coef*scale_sp*lmd/6 and scale_base*lmd)
  h1 = tanh(y); h2 = tanh(h1 @ blockdiag(W1) + b1); logits = <h2,W2>_head + b2
"""
import sys, types
import numpy as np

B, I, O, H, NB = 1024, 784, 64, 10, 8
NC = 8
BC = B // NC      # 128
P = 112
CH = I // P       # 7
HO = H * O        # 640
D2 = H * 32       # 320
NH = 2            # PSUM halves of HO
JSPLIT = 5        # weight dma piece A covers j<JSPLIT, piece B the rest


def _install_ntff_hook():
    if "antenv.axon_hooks" in sys.modules:
        return
    try:
        import antenv
        mod = types.ModuleType("antenv.axon_hooks")
        _h = [None]
        mod.set_axon_ntff_profile_hook = lambda h: _h.__setitem__(0, h)
        mod.get_axon_ntff_profile_hook = lambda: _h[0]
        sys.modules["antenv.axon_hooks"] = mod
        antenv.axon_hooks = mod
        from trn_agent_boot.trn_boot import _ntff_profile_via_ctypes
        h = _ntff_profile_via_ctypes("/opt/axon/libaxon_pjrt.so")
        if h is not None:
            mod.set_axon_ntff_profile_hook(h)
    except Exception:
        pass


_CACHE = {}


def _build():
    if "nc" in _CACHE:
        return _CACHE["nc"]
    import concourse.bacc as bacc
    import concourse.bass as bass
    import concourse.tile as tile
    from concourse import mybir
    from contextlib import ExitStack

    f32, f16, i32 = mybir.dt.float32, mybir.dt.float16, mybir.dt.int32
    ALU = mybir.AluOpType
    AF = mybir.ActivationFunctionType

    nc = bacc.Bacc("TRN2", target_bir_lowering=False, debug=False)
    x_d = nc.dram_tensor("x", (P, CH, BC), f32, kind="ExternalInput").ap()
    w_d = nc.dram_tensor("w", (CH, P, NB + 1, HO), f16, kind="ExternalInput").ap()
    w1_d = nc.dram_tensor("w1", (128, 5, D2), f16, kind="ExternalInput").ap()
    id_d = nc.dram_tensor("ident", (128, 128), f16, kind="ExternalInput").ap()
    b1_d = nc.dram_tensor("b1", (D2,), f32, kind="ExternalInput").ap()
    w2_d = nc.dram_tensor("w2", (D2,), f32, kind="ExternalInput").ap()
    b2_d = nc.dram_tensor("b2", (H,), f32, kind="ExternalInput").ap()
    out_d = nc.dram_tensor("out", (BC, H), f32, kind="ExternalOutput").ap()

    def bcast(dram_ap, n):
        return bass.AP(tensor=dram_ap.tensor, offset=dram_ap.offset,
                       ap=[[0, 128]] + [[1, n]])

    with tile.TileContext(nc) as tc, ExitStack() as ctx:
        sb = ctx.enter_context(tc.tile_pool(name="sb", bufs=1))
        ps = ctx.enter_context(tc.tile_pool(name="ps", bufs=1, space="PSUM"))

        # ---- x first (gates everything): split across both HWDGE queues ----
        xt = sb.tile([P, CH, BC], f32, tag="xt")
        nc.sync.dma_start(xt[:, 0:4, :], x_d[:, 0:4, :])
        nc.scalar.dma_start(xt[:, 4:CH, :], x_d[:, 4:CH, :])

        # ---- weights: per-chunk contiguous pieces, j-split, 3 queue engines ----
        wc = []
        qeng = [nc.sync, nc.scalar, nc.gpsimd]
        for c in range(CH):
            t = sb.tile([P, NB + 1, HO], f16, tag=f"wc{c}")
            qeng[c % 3].dma_start(t[:, 0:JSPLIT, :], w_d[c, :, 0:JSPLIT, :])
            wc.append(t)
        for c in range(CH):
            qeng[(c + 1) % 3].dma_start(
                wc[c][:, JSPLIT:NB + 1, :], w_d[c, :, JSPLIT:NB + 1, :])
        w1t = sb.tile([128, 5, D2], f16, tag="w1t")
        nc.gpsimd.dma_start(w1t[:], w1_d)
        idt = sb.tile([128, 128], f16, tag="idt")
        nc.gpsimd.dma_start(idt[:], id_d)
        b1b = sb.tile([128, D2], f32, tag="b1b")
        nc.gpsimd.dma_start(b1b[:], bcast(b1_d, D2))
        w2b = sb.tile([128, D2], f32, tag="w2b")
        nc.gpsimd.dma_start(w2b[:], bcast(w2_d, D2))
        b2b = sb.tile([128, H], f32, tag="b2b")
        nc.gpsimd.dma_start(b2b[:], bcast(b2_d, H))

        x = xt[:].rearrange("p c b -> p (c b)")

        def T(tag, dt=f16):
            return sb.tile([P, CH * BC], dt, tag=tag, name=tag)

        # ---- interval index t = floor(5x) via round(5x-0.5); masks; u ----
        ti = T("ti", i32)
        nc.vector.tensor_scalar(ti[:], x, 5.0, -0.5, op0=ALU.mult, op1=ALU.add)
        m = []
        for t in range(5):
            mt = T(f"m{t}")
            nc.vector.tensor_scalar(mt[:], ti[:], t, None, op0=ALU.is_equal)
            m.append(mt)
        tf = T("tf", f32)
        nc.vector.tensor_copy(tf[:], ti[:])
        u = T("u", f32)
        nc.vector.scalar_tensor_tensor(u[:], x, 5.0, tf[:],
                                       op0=ALU.mult, op1=ALU.subtract)

        # ---- local cubics (x6): p0=(1-u)^3, p1=3u^3-6u^2+4=(3u-6)u^2+4,
        #      p2=p1(1-u), p3=u^3 ----
        u2 = T("u2", f32); nc.scalar.activation(u2[:], u[:], AF.Square)
        w_ = T("w_", f32)
        nc.scalar.activation(w_[:], u[:], AF.Copy, bias=1.0, scale=-1.0)
        w2_ = T("w2_", f32); nc.scalar.activation(w2_[:], w_[:], AF.Square)
        a_ = T("a_", f32)
        nc.scalar.activation(a_[:], u[:], AF.Copy, bias=-6.0, scale=3.0)
        b_ = T("b_", f32)
        nc.scalar.activation(b_[:], w_[:], AF.Copy, bias=-6.0, scale=3.0)
        p3h = T("p3h"); nc.vector.tensor_tensor(p3h[:], u2[:], u[:], op=ALU.mult)
        p0h = T("p0h"); nc.vector.tensor_tensor(p0h[:], w2_[:], w_[:], op=ALU.mult)
        p1pre = T("p1pre", f32)
        nc.vector.tensor_tensor(p1pre[:], a_[:], u2[:], op=ALU.mult)
        p1h = T("p1h")
        nc.scalar.activation(p1h[:], p1pre[:], AF.Copy, bias=4.0, scale=1.0)
        p2pre = T("p2pre", f32)
        nc.vector.tensor_tensor(p2pre[:], b_[:], w2_[:], op=ALU.mult)
        p2h = T("p2h")
        nc.scalar.activation(p2h[:], p2pre[:], AF.Copy, bias=4.0, scale=1.0)
        ph = [p0h, p1h, p2h, p3h]

        # ---- features ----
        f_ = []
        for j in range(NB):
            f_.append(sb.tile([P, CH, BC], f16, tag=f"f{j}", name=f"f{j}"))
        fs = sb.tile([P, CH, BC], f16, tag="f8")
        nc.scalar.activation(fs[:].rearrange("p c b -> p (c b)"), x, AF.Silu)
        f_.append(fs)

        psum = [ps.tile([128, D2], f32, tag=f"y{nh}", name=f"y{nh}")
                for nh in range(NH)]
        nmm = [0, 0]

        def emit_mms(j):
            for c in range(CH):
                for nh in range(NH):
                    nc.tensor.matmul(
                        psum[nh][:],
                        f_[j][:, c, :],
                        wc[c][:, j, nh * D2:(nh + 1) * D2],
                        start=(nmm[nh] == 0),
                        stop=(nmm[nh] == CH * (NB + 1) - 1),
                    )
                    nmm[nh] += 1

        tmp = T("tmp")
        tmp2 = T("tmp2")
        for j in (0, 7, 1, 6, 2, 5, 3, 4):
            terms = [(t, j - t) for t in range(5) if 0 <= j - t <= 3]
            out = f_[j][:].rearrange("p c b -> p (c b)")
            if len(terms) == 1:
                t, r = terms[0]
                nc.vector.tensor_tensor(out, m[t][:], ph[r][:], op=ALU.mult)
            else:
                acc = tmp[:]
                t, r = terms[0]
                nc.vector.tensor_tensor(acc, m[t][:], ph[r][:], op=ALU.mult)
                for k, (t, r) in enumerate(terms[1:]):
                    pr = tmp2[:]
                    nc.vector.tensor_tensor(pr, m[t][:], ph[r][:], op=ALU.mult)
                    dst = out if k == len(terms) - 2 else acc
                    nc.vector.tensor_tensor(dst, acc, pr, op=ALU.add)
            emit_mms(j)
        emit_mms(NB)  # silu feature

        # ---- tail ----
        h1 = sb.tile([128, HO], f16, tag="h1")
        for nh in range(NH):
            nc.scalar.activation(h1[:, nh * D2:(nh + 1) * D2], psum[nh][:], AF.Tanh)
        h1t = []
        for k in range(5):
            pt = ps.tile([128, 128], f16, tag=f"pt{k}", name=f"pt{k}")
            nc.tensor.transpose(pt[:], h1[:, k * 128:(k + 1) * 128], idt[:])
            st = sb.tile([128, 128], f16, tag=f"h1t{k}", name=f"h1t{k}")
            nc.vector.tensor_copy(st[:], pt[:])
            h1t.append(st)
        ps2 = ps.tile([128, D2], f32, tag="ps2")
        for k in range(5):
            nc.tensor.matmul(ps2[:], h1t[k][:], w1t[:, k, :],
                             start=(k == 0), stop=(k == 4))
        h2a = sb.tile([128, D2], f32, tag="h2a")
        nc.vector.tensor_tensor(h2a[:], ps2[:], b1b[:], op=ALU.add)
        h2 = sb.tile([128, D2], f32, tag="h2")
        nc.scalar.activation(h2[:], h2a[:], AF.Tanh)
        prod = sb.tile([128, D2], f32, tag="prod")
        nc.vector.tensor_tensor(prod[:], h2[:], w2b[:], op=ALU.mult)
        red = sb.tile([128, H], f32, tag="red")
        nc.vector.tensor_reduce(red[:], prod[:].rearrange("p (h d) -> p h d", d=32),
                                axis=mybir.AxisListType.X, op=ALU.add)
        lg = sb.tile([128, H], f32, tag="lg")
        nc.vector.tensor_tensor(lg[:], red[:], b2b[:], op=ALU.add)
        nc.sync.dma_start(out_d, lg[:])

    nc.compile()
    _CACHE["nc"] = nc
    return nc


def _prep_inputs(x, coef, scale_base, scale_sp, lmd, W1, b1, W2, b2):
    xf = np.asarray(x, np.float64).reshape(B, I)
    coef = np.asarray(coef, np.float64)
    eff = coef * np.asarray(scale_sp, np.float64)[..., None] \
        * np.asarray(lmd, np.float64)[:, :, None, None] / 6.0
    sbl = np.asarray(scale_base, np.float64) \
        * np.asarray(lmd, np.float64)[:, :, None]
    wbig = np.concatenate([eff, sbl[..., None]], -1)               # (H,I,O,9)
    wdev = np.ascontiguousarray(
        wbig.reshape(H, CH, P, O, NB + 1).transpose(1, 2, 4, 0, 3)
    ).astype(np.float16)                                           # (CH,P,9,HO)
    W1 = np.asarray(W1, np.float64)
    w1bd = np.zeros((HO, D2))
    for h in range(H):
        w1bd[h * O:(h + 1) * O, h * 32:(h + 1) * 32] = W1[h]
    w1dev = np.ascontiguousarray(
        w1bd.reshape(5, 128, D2).transpose(1, 0, 2)).astype(np.float16)
    b1c = np.asarray(b1, np.float32).reshape(D2).copy()
    w2c = np.asarray(W2, np.float32).reshape(D2).copy()
    b2c = np.asarray(b2, np.float32).reshape(H).copy()
    ident = np.eye(128, dtype=np.float16)

    in_maps = []
    for core in range(NC):
        xs = xf[core * BC:(core + 1) * BC].T
        xdev = np.ascontiguousarray(
            xs.reshape(CH, P, BC).transpose(1, 0, 2)).astype(np.float32)
        in_maps.append({"x": xdev, "w": wdev, "w1": w1dev, "ident": ident,
                        "b1": b1c, "w2": w2c, "b2": b2c})
    return in_maps


def run(inputs, trace=False, tmpdir=None):
    _install_ntff_hook()
    from concourse.bass_utils import run_bass_kernel_spmd
    nc = _build()
    in_maps = _prep_inputs(**inputs)
    res = run_bass_kernel_spmd(nc, in_maps, core_ids=list(range(NC)),
                               trace=trace, tmpdir=tmpdir)
    out = np.concatenate([r["out"] for r in res.results], 0)
    return out.astype(np.float32), res


def kernel(**inputs):
    out, _ = run(inputs)
    return out
